# revision 15
# baseline (speedup 1.0000x reference)
"""Trainium2 Bass kernel for nn_Attention_11527692222464 (GAT-style attention).

Key algebraic restructuring (validated vs reference at ~6e-7 rel err):
  - Wh = h @ conv_w[h].T + conv_b  is needed densely only for the output stage.
  - The (N,N) score matrix is rank-1 + bias:
        score[b,h,i,j] = leaky(r[b,h,i] + c[b,h,j] + maskneg[b,i,j]) + a_bias[h,i,j]
    with r = h.v1 + const1 + Wh1_bias + Wh2_bias,  c = h.v2 + const2,
    maskneg = -1e10 where adj < 0.5 (leaky(-1e10) = -2e9 -> exp == 0).
  - Only softmax row-sums S and the diagonal are needed (the attention matrix
    is only consumed through its diagonal); softmax max-subtraction is skipped
    (unmasked scores are bounded by ~3.5).

Sharding: each of the 8 cores owns 256 rows (i) of the score matrix for all
(b,h); no collectives needed.

fp32 matmuls run at 1/4 rate on the PE (LOW_HIGH double pass), so the score
broadcasts run in bf16 where exact: identity and one-hot selectors are exact,
maskneg is exact in bf16, and c is split hi+lo into two bf16 rows stacked in
one K=16 matmul (exact to ~1e-5).  The tiny O(B*H*N) vectors r and c are
precomputed on the host (0.6% of FLOPs); the dense work (score matrix,
softmax stats, Wh matmul, output stage) all runs on device.

Device pipeline per (rt, h, b, jj) unit over a [128, 1024] tile:
  PSUM  <- sel2_bf16 x [c_hi; c_lo]  +  I_bf16 @ maskneg_bf16     (PE)
  t     <- Prelu(PSUM + r, alpha=0.2)        (ACT)
  u     <- t + a_bias                        (DVE tensor_tensor)
  p     <- Exp(u), accum_out -> row sums     (ACT)
Diagonal + output stage (elu(att*Wh + attention_bias)) on batched tiles.
"""

import numpy as np

import concourse.bacc as bacc
import concourse.bass as bass
import concourse.mybir as mybir
import concourse.tile as tile
from concourse import bass_utils

B, N, I, O, H = 4, 2048, 256, 128, 8
NC = 8
RPC = N // NC          # rows per core = 256
RT = RPC // 128        # row tiles per core = 2
P = 128
NEG = -1e10
FP = mybir.dt.float32
BF = mybir.dt.bfloat16
AF = mybir.ActivationFunctionType
ALU = mybir.AluOpType

_cached = None


def _build_kernel():
    nc = bacc.Bacc("TRN2", target_bir_lowering=False, debug=False, num_devices=NC)

    def din(name, shape, dt=FP):
        return nc.dram_tensor(name, list(shape), dt, kind="ExternalInput").ap()

    d = {}
    d["hTo"] = din("hTo", (P, 2048))           # own-rows hT: [k, (b*2+kt)*256+il]
    d["adjr"] = din("adjr", (B, RT, P, N))     # own adj rows
    d["abr"] = din("abr", (H, RT, P, N))       # own a_bias rows
    d["chl"] = din("chl", (16, B * N), BF)     # c hi (rows 0-7) / lo (8-15)
    d["sel2"] = din("sel2", (16, H * P), BF)   # one-hot selectors (hi+lo)
    d["identb"] = din("identb", (P, P), BF)    # identity, bf16
    d["ones1b"] = din("ones1b", (1, P), BF)
    d["cwTr"] = din("cwTr", (P, 2048))         # conv_w^T [k, (h*2+kt)*128+o]
    d["cbh"] = din("cbh", (1, H * P), BF)      # conv_b hi
    d["cbl"] = din("cbl", (1, H * P), BF)      # conv_b lo
    d["wsd"] = din("wsd", (P, RT * B * 16))    # r~/c~ at own rows (biases in)
    d["abdw"] = din("abdw", (P, 64))           # a_bias diagonal
    d["adjdw"] = din("adjdw", (P, 64))         # adj diagonal
    d["attbT"] = din("attbT", (P, 2048))       # attention_bias [p, rt*1024+h*128+o]
    d["out"] = nc.dram_tensor("out", [B, RT, P, H * O], FP,
                              kind="ExternalOutput").ap()

    with tile.TileContext(nc) as tc:
        _body(tc, d)

    nc.compile()
    return nc


def _body(tc, d):
    from contextlib import ExitStack
    nc = tc.nc
    ctx = ExitStack()
    with ctx:
        const = ctx.enter_context(tc.tile_pool(name="const", bufs=1))
        abp = ctx.enter_context(tc.tile_pool(name="abp", bufs=2))
        maskp = ctx.enter_context(tc.tile_pool(name="maskp", bufs=5))
        adjp = ctx.enter_context(tc.tile_pool(name="adjp", bufs=2))
        stp = ctx.enter_context(tc.tile_pool(name="stp", bufs=8))
        tp = ctx.enter_context(tc.tile_pool(name="tp", bufs=3))
        up = ctx.enter_context(tc.tile_pool(name="up", bufs=3))
        scr = ctx.enter_context(tc.tile_pool(name="scr", bufs=3))
        dgp = ctx.enter_context(tc.tile_pool(name="dgp", bufs=16))
        outp = ctx.enter_context(tc.tile_pool(name="outp", bufs=4))
        osm = ctx.enter_context(tc.tile_pool(name="osm", bufs=2))
        pscore = ctx.enter_context(tc.tile_pool(name="pscore", bufs=3, space="PSUM"))
        pwh = ctx.enter_context(tc.tile_pool(name="pwh", bufs=1, space="PSUM"))

        def cload(name, dt=FP):
            ap = d[name]
            t = const.tile(list(ap.shape), dt, name=name)
            nc.sync.dma_start(t[:], ap)
            return t

        hTo = cload("hTo")
        chl = cload("chl", BF)
        sel2 = cload("sel2", BF)
        identb = cload("identb", BF)
        ones1b = cload("ones1b", BF)
        cwTr = cload("cwTr")
        cbh = cload("cbh", BF)
        cbl = cload("cbl", BF)
        wsd = cload("wsd")
        abdw = cload("abdw")
        adjdw = cload("adjdw")
        attbT = cload("attbT")

        for rt in range(RT):
            mask = {}
            s_tmp = {}
            out_sb = {}
            for b in range(B):
                adj_t = adjp.tile([P, N], FP, tag="adj", name="adj_t")
                nc.sync.dma_start(adj_t[:], d["adjr"][b, rt])
                m = maskp.tile([P, N], BF, tag="mask", name="m")
                nc.vector.tensor_scalar(m[:], adj_t[:], 0.5, NEG,
                                        ALU.is_lt, ALU.mult)
                mask[b] = m
                s_tmp[b] = stp.tile([P, 16], FP, tag="stmp", name="s_tmp")
                out_sb[b] = outp.tile([P, H * O], FP, tag="outsb", name="out_sb")

            for hh in range(H):
                ab_t = abp.tile([P, N], FP, tag="ab", name="ab_t")
                nc.sync.dma_start(ab_t[:], d["abr"][hh, rt])
                for b in range(B):
                    selc = sel2[:, hh * P:(hh + 1) * P]
                    rcol = wsd[:, (rt * B + b) * 16 + 2 * hh:
                               (rt * B + b) * 16 + 2 * hh + 1]
                    for jj in range(2):
                        ps = pscore.tile([P, 1024], FP, tag="score", name="ps")
                        # same stationary weights for both chunks (fewer
                        # LDWEIGHTS); accumulation groups are per psum bank
                        for q in range(2):
                            j0 = jj * 1024 + q * 512
                            nc.tensor.matmul(ps[:, q * 512:(q + 1) * 512],
                                             selc,
                                             chl[:, b * N + j0:b * N + j0 + 512],
                                             start=True, stop=False)
                        for q in range(2):
                            j0 = jj * 1024 + q * 512
                            nc.tensor.matmul(ps[:, q * 512:(q + 1) * 512],
                                             identb[:],
                                             mask[b][:, j0:j0 + 512],
                                             start=False, stop=True)
                        t_t = tp.tile([P, 1024], FP, tag="t", name="t_t")
                        nc.scalar.activation(t_t[:], ps[:], AF.Prelu,
                                             bias=rcol, scale=1.0, alpha=0.2)
                        u_t = up.tile([P, 1024], FP, tag="u", name="u_t")
                        nc.vector.tensor_add(
                            u_t[:], t_t[:],
                            ab_t[:, jj * 1024:(jj + 1) * 1024])
                        sc = scr.tile([P, 1024], FP, tag="scratch", name="sc")
                        nc.scalar.activation(
                            sc[:], u_t[:], AF.Exp, bias=0.0, scale=1.0,
                            accum_out=s_tmp[b][:, 2 * hh + jj:2 * hh + jj + 1])

            for b in range(B):
                # diagonal attention:  att_ii = p_ii / S_i
                dcol = (b * 2 + rt) * 8
                wcol = (rt * B + b) * 16
                S8 = dgp.tile([P, H], FP, tag="dg", name="S8")
                nc.vector.tensor_add(S8[:], s_tmp[b][:, 0:16:2],
                                     s_tmp[b][:, 1:16:2])
                xd = dgp.tile([P, H], FP, tag="dg", name="xd")
                nc.vector.tensor_add(xd[:], wsd[:, wcol:wcol + 16:2],
                                     wsd[:, wcol + 1:wcol + 16:2])
                mn = dgp.tile([P, H], FP, tag="dg", name="mn")
                nc.vector.tensor_scalar(mn[:], adjdw[:, dcol:dcol + 8], 0.5,
                                        NEG, ALU.is_lt, ALU.mult)
                td = dgp.tile([P, H], FP, tag="dg", name="td")
                nc.scalar.activation(td[:], xd[:], AF.Prelu, bias=0.0,
                                     scale=1.0, alpha=0.2)
                ed = dgp.tile([P, H], FP, tag="dg", name="ed")
                nc.vector.tensor_add(ed[:], td[:], abdw[:, dcol:dcol + 8])
                ed2 = dgp.tile([P, H], FP, tag="dg", name="ed2")
                nc.vector.tensor_add(ed2[:], ed[:], mn[:])
                pd = dgp.tile([P, H], FP, tag="dg", name="pd")
                nc.scalar.activation(pd[:], ed2[:], AF.Exp, bias=0.0, scale=1.0)
                sr = dgp.tile([P, H], FP, tag="dg", name="sr")
                nc.vector.reciprocal(sr[:], S8[:])
                att = dgp.tile([P, H], FP, tag="dg", name="att")
                nc.vector.tensor_mul(att[:], pd[:], sr[:])

                # output stage: out = elu(att * (h @ conv_w.T + conv_b) + attb)
                col0 = (b * 2 + 0) * 256 + rt * 128
                col1 = (b * 2 + 1) * 256 + rt * 128
                wq = pwh.tile([P, H * O], FP, tag="wh", name="wq")
                for kt, c0 in ((0, col0), (1, col1)):
                    for hh in range(H):
                        # start=True clears has_written for the WHOLE bank, so
                        # it must fire exactly once per 512-col bank (hh 0, 4)
                        nc.tensor.matmul(
                            wq[:, hh * O:(hh + 1) * O],
                            hTo[:, c0:c0 + 128],
                            cwTr[:, (hh * 2 + kt) * O:(hh * 2 + kt + 1) * O],
                            start=(kt == 0 and hh % 4 == 0), stop=False)
                for q in range(2):
                    nc.tensor.matmul(wq[:, q * 512:(q + 1) * 512], ones1b[:],
                                     cbh[:, q * 512:(q + 1) * 512],
                                     start=False, stop=False)
                    nc.tensor.matmul(wq[:, q * 512:(q + 1) * 512], ones1b[:],
                                     cbl[:, q * 512:(q + 1) * 512],
                                     start=False, stop=True)
                v = osm.tile([P, H * O], FP, tag="v", name="v")
                for hh in range(H):
                    nc.vector.tensor_scalar(v[:, hh * O:(hh + 1) * O],
                                            wq[:, hh * O:(hh + 1) * O],
                                            att[:, hh:hh + 1], None, ALU.mult)
                u = osm.tile([P, H * O], FP, tag="u2", name="u")
                nc.vector.tensor_add(u[:], v[:],
                                     attbT[:, rt * 1024:(rt + 1) * 1024])
                z = osm.tile([P, H * O], FP, tag="z", name="z")
                nc.vector.tensor_scalar(z[:], u[:], 0.0, -1.0, ALU.max, ALU.add)
                em = osm.tile([P, H * O], FP, tag="v", name="em")
                nc.vector.tensor_scalar(em[:], u[:], 0.0, None, ALU.min)
                ee = osm.tile([P, H * O], FP, tag="ee", name="ee")
                nc.scalar.activation(ee[:], em[:], AF.Exp, bias=0.0, scale=1.0)
                nc.vector.tensor_add(out_sb[b][:], z[:], ee[:])
                nc.sync.dma_start(d["out"][b, rt], out_sb[b][:])


def _host_prep(inputs):
    import ml_dtypes
    h = np.ascontiguousarray(np.asarray(inputs["h"], dtype=np.float32))
    adj = np.asarray(inputs["adj"], dtype=np.float32)
    conv_w = np.asarray(inputs["conv_w"], dtype=np.float32)
    conv_b = np.asarray(inputs["conv_b"], dtype=np.float32)
    a = np.asarray(inputs["a"], dtype=np.float32)
    Wh1b = np.asarray(inputs["Wh1_bias"], dtype=np.float32)
    Wh2b = np.asarray(inputs["Wh2_bias"], dtype=np.float32)
    ab = np.asarray(inputs["a_bias"], dtype=np.float32)
    attb = np.asarray(inputs["attention_bias"], dtype=np.float32)

    a1, a2 = a[:, :O], a[:, O:]
    v1 = np.einsum("hoi,ho->hi", conv_w, a1).astype(np.float32)
    v2 = np.einsum("hoi,ho->hi", conv_w, a2).astype(np.float32)
    c1 = np.einsum("ho,ho->h", conv_b, a1).astype(np.float32)
    c2 = np.einsum("ho,ho->h", conv_b, a2).astype(np.float32)

    # c[b,h,j] (+const2), bf16 hi/lo split, stacked [16, B*N]
    cfull = (np.einsum("bji,hi->bhj", h, v2)
             + c2[None, :, None]).astype(np.float32)
    chi = cfull.astype(ml_dtypes.bfloat16)
    clo = (cfull - chi.astype(np.float32)).astype(ml_dtypes.bfloat16)
    chl = np.empty((16, B * N), dtype=ml_dtypes.bfloat16)
    chl[0:8] = chi.transpose(1, 0, 2).reshape(H, B * N)
    chl[8:16] = clo.transpose(1, 0, 2).reshape(H, B * N)

    sel2 = np.zeros((16, H * P), dtype=ml_dtypes.bfloat16)
    for hh in range(H):
        sel2[hh, hh * P:(hh + 1) * P] = 1.0
        sel2[8 + hh, hh * P:(hh + 1) * P] = 1.0
    identb = np.eye(P, dtype=ml_dtypes.bfloat16)
    ones1b = np.ones((1, P), dtype=ml_dtypes.bfloat16)
    cb_row = conv_b.reshape(1, H * P).astype(np.float32)
    cbh = cb_row.astype(ml_dtypes.bfloat16)
    cbl = (cb_row - cbh.astype(np.float32)).astype(ml_dtypes.bfloat16)
    cwTr = np.ascontiguousarray(
        conv_w.transpose(2, 0, 1).reshape(2, P, H, O)
        .transpose(1, 2, 0, 3).reshape(P, 2048))
    ab_diag = np.ascontiguousarray(np.einsum("hnn->hn", ab))   # (H, N)
    adj_diag = np.ascontiguousarray(np.einsum("bnn->bn", adj))  # (B, N)

    # r~ (with all biases) at all rows; sliced per core below
    rfull = (np.einsum("bji,hi->bhj", h, v1) + c1[None, :, None]
             + (Wh1b[:, :, 0] + Wh2b[:, :, 0])[None]).astype(np.float32)

    shared = dict(chl=chl, sel2=sel2, identb=identb, ones1b=ones1b,
                  cwTr=cwTr, cbh=cbh, cbl=cbl)

    in_maps = []
    for k in range(NC):
        k0 = k * RPC
        rows = slice(k0, k0 + RPC)
        hTo = np.ascontiguousarray(
            h[:, rows, :].transpose(2, 0, 1).reshape(2, P, B, RPC)
            .transpose(1, 2, 0, 3).reshape(P, 2048))
        adjr = np.ascontiguousarray(adj[:, rows, :]).reshape(B, RT, P, N)
        abr = np.ascontiguousarray(ab[:, rows, :]).reshape(H, RT, P, N)
        wsd = np.empty((P, RT * B * 16), dtype=np.float32)
        abdw = np.empty((P, 64), dtype=np.float32)
        adjdw = np.empty((P, 64), dtype=np.float32)
        for rt in range(RT):
            rsl = slice(k0 + rt * P, k0 + (rt + 1) * P)
            for b in range(B):
                wcol = (rt * B + b) * 16
                wsd[:, wcol:wcol + 16:2] = rfull[b][:, rsl].T
                wsd[:, wcol + 1:wcol + 16:2] = cfull[b][:, rsl].T
                dcol = (b * 2 + rt) * 8
                abdw[:, dcol:dcol + 8] = ab_diag[:, rsl].T
                adjdw[:, dcol:dcol + 8] = adj_diag[b, rsl][:, None]
        attbT = np.ascontiguousarray(
            attb[:, rows, :].transpose(1, 0, 2).reshape(RT, P, H * O)
            .transpose(1, 0, 2).reshape(P, RT * H * O))
        m = dict(shared)
        m.update(hTo=hTo, adjr=adjr, abr=abr, wsd=wsd, abdw=abdw,
                 adjdw=adjdw, attbT=attbT)
        in_maps.append(m)
    return in_maps


def kernel(**inputs) -> np.ndarray:
    global _cached
    if _cached is None:
        _cached = _build_kernel()
    nc = _cached
    in_maps = _host_prep(inputs)
    res = bass_utils.run_bass_kernel_spmd(nc, in_maps, core_ids=list(range(NC)))
    out = np.empty((B, N, H * O), dtype=np.float32)
    for k in range(NC):
        o = res.results[k]["out"]          # (B, RT, P, H*O)
        out[:, k * RPC:(k + 1) * RPC, :] = o.reshape(B, RPC, H * O)
    return out


# revision 16
# speedup vs baseline: 1.0907x; 1.0907x over previous
"""Trainium2 Bass kernel for nn_Attention_11527692222464 (GAT-style attention).

Key algebraic restructuring (validated vs reference at ~6e-7 rel err):
  - Wh = h @ conv_w[h].T + conv_b  is needed densely only for the output stage.
  - The (N,N) score matrix is rank-1 + bias:
        score[b,h,i,j] = leaky(r[b,h,i] + c[b,h,j] + maskneg[b,i,j]) + a_bias[h,i,j]
    with r = h.v1 + const1 + Wh1_bias + Wh2_bias,  c = h.v2 + const2,
    maskneg = -1e10 where adj < 0.5 (leaky(-1e10) = -2e9 -> exp == 0).
  - Only softmax row-sums S and the diagonal are needed (the attention matrix
    is only consumed through its diagonal); softmax max-subtraction is skipped
    (unmasked scores are bounded by ~3.5).

Sharding: each of the 8 cores owns 256 rows (i) of the score matrix for all
(b,h); no collectives needed.

fp32 matmuls run at 1/4 rate on the PE (LOW_HIGH double pass), so the score
broadcasts run in bf16 where exact: identity and one-hot selectors are exact,
maskneg is exact in bf16, and c is split hi+lo into two bf16 rows stacked in
one K=16 matmul (exact to ~1e-5).  The tiny O(B*H*N) vectors r and c are
precomputed on the host (0.6% of FLOPs); the dense work (score matrix,
softmax stats, Wh matmul, output stage) all runs on device.

Device pipeline per (rt, h, b, jj) unit over a [128, 1024] tile:
  PSUM  <- sel2_bf16 x [c_hi; c_lo]  +  I_bf16 @ maskneg_bf16     (PE)
  t     <- Prelu(PSUM + r, alpha=0.2)        (ACT)
  u     <- t + a_bias                        (DVE tensor_tensor)
  p     <- Exp(u), accum_out -> row sums     (ACT)
Diagonal + output stage (elu(att*Wh + attention_bias)) on batched tiles.
"""

import numpy as np

import concourse.bacc as bacc
import concourse.bass as bass
import concourse.mybir as mybir
import concourse.tile as tile
from concourse import bass_utils

B, N, I, O, H = 4, 2048, 256, 128, 8
NC = 8
RPC = N // NC          # rows per core = 256
RT = RPC // 128        # row tiles per core = 2
P = 128
NEG = -1e10
FP = mybir.dt.float32
BF = mybir.dt.bfloat16
AF = mybir.ActivationFunctionType
ALU = mybir.AluOpType

_cached = None


def _build_kernel():
    nc = bacc.Bacc("TRN2", target_bir_lowering=False, debug=False, num_devices=NC)

    def din(name, shape, dt=FP):
        return nc.dram_tensor(name, list(shape), dt, kind="ExternalInput").ap()

    d = {}
    d["hTo"] = din("hTo", (P, 2048))           # own-rows hT: [k, (b*2+kt)*256+il]
    d["adjr"] = din("adjr", (B, RT, P, N))     # own adj rows
    d["abr"] = din("abr", (H, RT, P, N))       # own a_bias rows
    d["chl"] = din("chl", (16, B * N), BF)     # c hi (rows 0-7) / lo (8-15)
    d["sel2"] = din("sel2", (16, H * P), BF)   # one-hot selectors (hi+lo)
    d["identb"] = din("identb", (P, P), BF)    # identity, bf16
    d["ones1b"] = din("ones1b", (1, P), BF)
    d["cwTr"] = din("cwTr", (P, 2048))         # conv_w^T [k, (h*2+kt)*128+o]
    d["cbh"] = din("cbh", (1, H * P), BF)      # conv_b hi
    d["cbl"] = din("cbl", (1, H * P), BF)      # conv_b lo
    d["wsd"] = din("wsd", (P, RT * B * 16))    # r~/c~ at own rows (biases in)
    d["abdw"] = din("abdw", (P, 64))           # a_bias diagonal
    d["adjdw"] = din("adjdw", (P, 64))         # adj diagonal
    d["attbT"] = din("attbT", (P, 2048))       # attention_bias [p, rt*1024+h*128+o]
    d["out"] = nc.dram_tensor("out", [B, RT, P, H * O], FP,
                              kind="ExternalOutput").ap()

    with tile.TileContext(nc) as tc:
        _body(tc, d)

    nc.compile()
    return nc


def _body(tc, d):
    from contextlib import ExitStack
    nc = tc.nc
    ctx = ExitStack()
    with ctx:
        const = ctx.enter_context(tc.tile_pool(name="const", bufs=1))
        abp = ctx.enter_context(tc.tile_pool(name="abp", bufs=2))
        maskp = ctx.enter_context(tc.tile_pool(name="maskp", bufs=5))
        adjp = ctx.enter_context(tc.tile_pool(name="adjp", bufs=2))
        stp = ctx.enter_context(tc.tile_pool(name="stp", bufs=8))
        tp = ctx.enter_context(tc.tile_pool(name="tp", bufs=2))
        up = ctx.enter_context(tc.tile_pool(name="up", bufs=2))
        scr = ctx.enter_context(tc.tile_pool(name="scr", bufs=2))
        dgp = ctx.enter_context(tc.tile_pool(name="dgp", bufs=16))
        outp = ctx.enter_context(tc.tile_pool(name="outp", bufs=4))
        osm = ctx.enter_context(tc.tile_pool(name="osm", bufs=2))
        pscore = ctx.enter_context(tc.tile_pool(name="pscore", bufs=2, space="PSUM"))

        def cload(name, dt=FP):
            ap = d[name]
            t = const.tile(list(ap.shape), dt, name=name)
            nc.sync.dma_start(t[:], ap)
            return t

        hTo = cload("hTo")
        chl = cload("chl", BF)
        sel2 = cload("sel2", BF)
        identb = cload("identb", BF)
        ones1b = cload("ones1b", BF)
        cwTr = cload("cwTr")
        cbh = cload("cbh", BF)
        cbl = cload("cbl", BF)
        wsd = cload("wsd")
        abdw = cload("abdw")
        adjdw = cload("adjdw")
        attbT = cload("attbT")

        for rt in range(RT):
            mask = {}
            s_tmp = {}
            out_sb = {}
            for b in range(B):
                adj_t = adjp.tile([P, N], FP, tag="adj", name="adj_t")
                nc.sync.dma_start(adj_t[:], d["adjr"][b, rt])
                m = maskp.tile([P, N], BF, tag="mask", name="m")
                nc.vector.tensor_scalar(m[:], adj_t[:], 0.5, NEG,
                                        ALU.is_lt, ALU.mult)
                mask[b] = m
                s_tmp[b] = stp.tile([P, H], FP, tag="stmp", name="s_tmp")
                out_sb[b] = outp.tile([P, H * O], FP, tag="outsb", name="out_sb")

            for hh in range(H):
                ab_t = abp.tile([P, N], FP, tag="ab", name="ab_t")
                nc.sync.dma_start(ab_t[:], d["abr"][hh, rt])
                for b in range(B):
                    selc = sel2[:, hh * P:(hh + 1) * P]
                    rcol = wsd[:, (rt * B + b) * 16 + 2 * hh:
                               (rt * B + b) * 16 + 2 * hh + 1]
                    ps = pscore.tile([P, N], FP, tag="score", name="ps")
                    # same stationary weights for all chunks (fewer
                    # LDWEIGHTS); accumulation groups are per psum bank
                    for q in range(4):
                        j0 = q * 512
                        nc.tensor.matmul(ps[:, j0:j0 + 512], selc,
                                         chl[:, b * N + j0:b * N + j0 + 512],
                                         start=True, stop=False)
                    for q in range(4):
                        j0 = q * 512
                        nc.tensor.matmul(ps[:, j0:j0 + 512], identb[:],
                                         mask[b][:, j0:j0 + 512],
                                         start=False, stop=True)
                    t_t = tp.tile([P, N], FP, tag="t", name="t_t")
                    nc.scalar.activation(t_t[:], ps[:], AF.Prelu,
                                         bias=rcol, scale=1.0, alpha=0.2)
                    u_t = up.tile([P, N], FP, tag="u", name="u_t")
                    nc.vector.tensor_add(u_t[:], t_t[:], ab_t[:])
                    sc = scr.tile([P, N], BF, tag="scratch", name="sc")
                    nc.scalar.activation(
                        sc[:], u_t[:], AF.Exp, bias=0.0, scale=1.0,
                        accum_out=s_tmp[b][:, hh:hh + 1])

            for b in range(B):
                # diagonal attention:  att_ii = p_ii / S_i
                dcol = (b * 2 + rt) * 8
                wcol = (rt * B + b) * 16
                xd = dgp.tile([P, H], FP, tag="dg", name="xd")
                nc.vector.tensor_add(xd[:], wsd[:, wcol:wcol + 16:2],
                                     wsd[:, wcol + 1:wcol + 16:2])
                mn = dgp.tile([P, H], FP, tag="dg", name="mn")
                nc.vector.tensor_scalar(mn[:], adjdw[:, dcol:dcol + 8], 0.5,
                                        NEG, ALU.is_lt, ALU.mult)
                td = dgp.tile([P, H], FP, tag="dg", name="td")
                nc.scalar.activation(td[:], xd[:], AF.Prelu, bias=0.0,
                                     scale=1.0, alpha=0.2)
                ed = dgp.tile([P, H], FP, tag="dg", name="ed")
                nc.vector.tensor_add(ed[:], td[:], abdw[:, dcol:dcol + 8])
                ed2 = dgp.tile([P, H], FP, tag="dg", name="ed2")
                nc.vector.tensor_add(ed2[:], ed[:], mn[:])
                pd = dgp.tile([P, H], FP, tag="dg", name="pd")
                nc.scalar.activation(pd[:], ed2[:], AF.Exp, bias=0.0, scale=1.0)
                sr = dgp.tile([P, H], FP, tag="dg", name="sr")
                nc.vector.reciprocal(sr[:], s_tmp[b][:])
                att = dgp.tile([P, H], FP, tag="dg", name="att")
                nc.vector.tensor_mul(att[:], pd[:], sr[:])

                # output stage: out = elu(att * (h @ conv_w.T + conv_b) + attb)
                col0 = (b * 2 + 0) * 256 + rt * 128
                col1 = (b * 2 + 1) * 256 + rt * 128
                wq = pscore.tile([P, H * O], FP, tag="score", name="wq")
                for kt, c0 in ((0, col0), (1, col1)):
                    for hh in range(H):
                        # start=True clears has_written for the WHOLE bank, so
                        # it must fire exactly once per 512-col bank (hh 0, 4)
                        nc.tensor.matmul(
                            wq[:, hh * O:(hh + 1) * O],
                            hTo[:, c0:c0 + 128],
                            cwTr[:, (hh * 2 + kt) * O:(hh * 2 + kt + 1) * O],
                            start=(kt == 0 and hh % 4 == 0), stop=False)
                for q in range(2):
                    nc.tensor.matmul(wq[:, q * 512:(q + 1) * 512], ones1b[:],
                                     cbh[:, q * 512:(q + 1) * 512],
                                     start=False, stop=False)
                    nc.tensor.matmul(wq[:, q * 512:(q + 1) * 512], ones1b[:],
                                     cbl[:, q * 512:(q + 1) * 512],
                                     start=False, stop=True)
                v = osm.tile([P, H * O], FP, tag="v", name="v")
                for hh in range(H):
                    nc.vector.tensor_scalar(v[:, hh * O:(hh + 1) * O],
                                            wq[:, hh * O:(hh + 1) * O],
                                            att[:, hh:hh + 1], None, ALU.mult)
                u = osm.tile([P, H * O], FP, tag="u2", name="u")
                nc.vector.tensor_add(u[:], v[:],
                                     attbT[:, rt * 1024:(rt + 1) * 1024])
                z = osm.tile([P, H * O], FP, tag="z", name="z")
                nc.vector.tensor_scalar(z[:], u[:], 0.0, -1.0, ALU.max, ALU.add)
                em = osm.tile([P, H * O], FP, tag="v", name="em")
                nc.vector.tensor_scalar(em[:], u[:], 0.0, None, ALU.min)
                ee = osm.tile([P, H * O], FP, tag="ee", name="ee")
                nc.scalar.activation(ee[:], em[:], AF.Exp, bias=0.0, scale=1.0)
                nc.vector.tensor_add(out_sb[b][:], z[:], ee[:])
                nc.sync.dma_start(d["out"][b, rt], out_sb[b][:])


def _host_prep(inputs):
    import ml_dtypes
    h = np.ascontiguousarray(np.asarray(inputs["h"], dtype=np.float32))
    adj = np.asarray(inputs["adj"], dtype=np.float32)
    conv_w = np.asarray(inputs["conv_w"], dtype=np.float32)
    conv_b = np.asarray(inputs["conv_b"], dtype=np.float32)
    a = np.asarray(inputs["a"], dtype=np.float32)
    Wh1b = np.asarray(inputs["Wh1_bias"], dtype=np.float32)
    Wh2b = np.asarray(inputs["Wh2_bias"], dtype=np.float32)
    ab = np.asarray(inputs["a_bias"], dtype=np.float32)
    attb = np.asarray(inputs["attention_bias"], dtype=np.float32)

    a1, a2 = a[:, :O], a[:, O:]
    v1 = np.einsum("hoi,ho->hi", conv_w, a1).astype(np.float32)
    v2 = np.einsum("hoi,ho->hi", conv_w, a2).astype(np.float32)
    c1 = np.einsum("ho,ho->h", conv_b, a1).astype(np.float32)
    c2 = np.einsum("ho,ho->h", conv_b, a2).astype(np.float32)

    # c[b,h,j] (+const2), bf16 hi/lo split, stacked [16, B*N]
    cfull = (np.einsum("bji,hi->bhj", h, v2)
             + c2[None, :, None]).astype(np.float32)
    chi = cfull.astype(ml_dtypes.bfloat16)
    clo = (cfull - chi.astype(np.float32)).astype(ml_dtypes.bfloat16)
    chl = np.empty((16, B * N), dtype=ml_dtypes.bfloat16)
    chl[0:8] = chi.transpose(1, 0, 2).reshape(H, B * N)
    chl[8:16] = clo.transpose(1, 0, 2).reshape(H, B * N)

    sel2 = np.zeros((16, H * P), dtype=ml_dtypes.bfloat16)
    for hh in range(H):
        sel2[hh, hh * P:(hh + 1) * P] = 1.0
        sel2[8 + hh, hh * P:(hh + 1) * P] = 1.0
    identb = np.eye(P, dtype=ml_dtypes.bfloat16)
    ones1b = np.ones((1, P), dtype=ml_dtypes.bfloat16)
    cb_row = conv_b.reshape(1, H * P).astype(np.float32)
    cbh = cb_row.astype(ml_dtypes.bfloat16)
    cbl = (cb_row - cbh.astype(np.float32)).astype(ml_dtypes.bfloat16)
    cwTr = np.ascontiguousarray(
        conv_w.transpose(2, 0, 1).reshape(2, P, H, O)
        .transpose(1, 2, 0, 3).reshape(P, 2048))
    ab_diag = np.ascontiguousarray(np.einsum("hnn->hn", ab))   # (H, N)
    adj_diag = np.ascontiguousarray(np.einsum("bnn->bn", adj))  # (B, N)

    # r~ (with all biases) at all rows; sliced per core below
    rfull = (np.einsum("bji,hi->bhj", h, v1) + c1[None, :, None]
             + (Wh1b[:, :, 0] + Wh2b[:, :, 0])[None]).astype(np.float32)

    shared = dict(chl=chl, sel2=sel2, identb=identb, ones1b=ones1b,
                  cwTr=cwTr, cbh=cbh, cbl=cbl)

    in_maps = []
    for k in range(NC):
        k0 = k * RPC
        rows = slice(k0, k0 + RPC)
        hTo = np.ascontiguousarray(
            h[:, rows, :].transpose(2, 0, 1).reshape(2, P, B, RPC)
            .transpose(1, 2, 0, 3).reshape(P, 2048))
        adjr = np.ascontiguousarray(adj[:, rows, :]).reshape(B, RT, P, N)
        abr = np.ascontiguousarray(ab[:, rows, :]).reshape(H, RT, P, N)
        wsd = np.empty((P, RT * B * 16), dtype=np.float32)
        abdw = np.empty((P, 64), dtype=np.float32)
        adjdw = np.empty((P, 64), dtype=np.float32)
        for rt in range(RT):
            rsl = slice(k0 + rt * P, k0 + (rt + 1) * P)
            for b in range(B):
                wcol = (rt * B + b) * 16
                wsd[:, wcol:wcol + 16:2] = rfull[b][:, rsl].T
                wsd[:, wcol + 1:wcol + 16:2] = cfull[b][:, rsl].T
                dcol = (b * 2 + rt) * 8
                abdw[:, dcol:dcol + 8] = ab_diag[:, rsl].T
                adjdw[:, dcol:dcol + 8] = adj_diag[b, rsl][:, None]
        attbT = np.ascontiguousarray(
            attb[:, rows, :].transpose(1, 0, 2).reshape(RT, P, H * O)
            .transpose(1, 0, 2).reshape(P, RT * H * O))
        m = dict(shared)
        m.update(hTo=hTo, adjr=adjr, abr=abr, wsd=wsd, abdw=abdw,
                 adjdw=adjdw, attbT=attbT)
        in_maps.append(m)
    return in_maps


def kernel(**inputs) -> np.ndarray:
    global _cached
    if _cached is None:
        _cached = _build_kernel()
    nc = _cached
    in_maps = _host_prep(inputs)
    res = bass_utils.run_bass_kernel_spmd(nc, in_maps, core_ids=list(range(NC)))
    out = np.empty((B, N, H * O), dtype=np.float32)
    for k in range(NC):
        o = res.results[k]["out"]          # (B, RT, P, H*O)
        out[:, k * RPC:(k + 1) * RPC, :] = o.reshape(B, RPC, H * O)
    return out
